# revision 7
# baseline (speedup 1.0000x reference)
"""Trainium2 Bass kernel for nn_AbstractAttention (B=2, S=2048, D=1024, H=16, dh=64).

Sharding: 8 cores = 2 batch groups x 4 cores. Core i handles batch i//4 and
heads 4*(i%4)..+4 for QKV projection + causal attention; z^T is AllGathered
(fp16, 1 MB/core) within each 4-core group and every core then runs the full
output projection; the host reads one core's output per batch.

Per core:
  - Q^T,K^T ([e,pos]) and V ([pos,e]) projections from DMA-transposed fp16
    inputs (host pre-casts to fp16; accumulation in fp32 PSUM).
  - per head: S^T[k,q] score tiles (only causal blocks), exp on ACT with the
    1/sqrt(dh) scale folded in, multiplicative triangular mask on diagonal
    blocks, unnormalized PV with a ones-column in V so z^T[64,:] is the
    softmax denominator; normalize via reciprocal + PE ones-broadcast.
  - AllGather z^T, out[2048,1024] = z_full^T.T @ W_O + b_O.
"""
import os, sys, types

sys.path.insert(0, "/opt/trn_rl_repo")
import numpy as np

import concourse.bass as bass
import concourse.bacc as bacc
import concourse.tile as tile
from concourse import mybir
from concourse.bass_utils import run_bass_kernel_spmd

B, S, D, H, DH = 2, 2048, 1024, 16, 64
N_CORES = 8
HPC = 4            # heads per core
QC = 512           # query chunk width for score tiles
NQC = S // QC      # 4
KB = 128           # key block
NKB = S // KB      # 16
NDMC = D // 128    # 8 d_model chunks
F16 = mybir.dt.float16
F32 = mybir.dt.float32
F32R = mybir.dt.float32r


def _install_ntff_hook():
    """Register the axon NTFF profiling hook missing from this image's antenv."""
    if "antenv.axon_hooks" in sys.modules:
        return
    try:
        from trn_agent_boot.trn_boot import _ntff_profile_via_ctypes

        hook = _ntff_profile_via_ctypes("/opt/axon/libaxon_pjrt.so")
        if hook is None:
            return
        import antenv  # noqa: F401

        mod = types.ModuleType("antenv.axon_hooks")
        mod.get_axon_ntff_profile_hook = lambda: hook
        sys.modules["antenv.axon_hooks"] = mod
    except Exception:
        pass


def build():
    nc = bacc.Bacc("TRN2", target_bir_lowering=False, debug=False, num_devices=N_CORES)
    xq = nc.dram_tensor("xq", [S, D], F16, kind="ExternalInput")
    xk = nc.dram_tensor("xk", [S, D], F16, kind="ExternalInput")
    xv = nc.dram_tensor("xv", [S, D], F16, kind="ExternalInput")
    wq = nc.dram_tensor("wq", [HPC, D, DH], F16, kind="ExternalInput")
    wk = nc.dram_tensor("wk", [HPC, D, DH], F16, kind="ExternalInput")
    wv = nc.dram_tensor("wv", [HPC, D, DH], F16, kind="ExternalInput")
    wo = nc.dram_tensor("wo", [H * DH, D], F16, kind="ExternalInput")
    bq = nc.dram_tensor("bq", [HPC, DH], F32, kind="ExternalInput")
    bk = nc.dram_tensor("bk", [HPC, DH], F32, kind="ExternalInput")
    bv = nc.dram_tensor("bv", [HPC, DH], F32, kind="ExternalInput")
    bo = nc.dram_tensor("bo", [D], F32, kind="ExternalInput")
    out = nc.dram_tensor("out", [S, D], F32, kind="ExternalOutput")

    tri_dram = nc.inline_tensor(np.triu(np.ones((128, 128), np.float16)), name="tri_c")
    ones_dram = nc.inline_tensor(np.ones((1, DH), np.float32), name="ones_c")

    with tile.TileContext(nc) as tc:
        with (
            tc.tile_pool(name="consts", bufs=1) as consts,
            tc.tile_pool(name="persist", bufs=1) as persist,
            tc.tile_pool(name="work", bufs=2) as work,
            tc.tile_pool(name="pt", bufs=3) as ptp,
            tc.tile_pool(name="zf", bufs=3) as zfp,
            tc.tile_pool(name="ps", bufs=1, space="PSUM") as ps,
            tc.tile_pool(name="ps2", bufs=2, space="PSUM") as ps2,
            tc.tile_pool(name="dram", bufs=1, space="DRAM") as dram,
        ):
            # ---- constants / weights / biases -------------------------------
            tri = consts.tile([128, 128], F16, tag="tri")
            nc.sync.dma_start(out=tri, in_=tri_dram.ap())

            wq_sb = consts.tile([128, NDMC, HPC, DH], F16, tag="wq")
            wk_sb = consts.tile([128, NDMC, HPC, DH], F16, tag="wk")
            wv_sb = consts.tile([128, NDMC, HPC, DH], F16, tag="wv")
            for dmc in range(NDMC):
                sl = slice(128 * dmc, 128 * (dmc + 1))
                nc.sync.dma_start(
                    out=wq_sb[:, dmc], in_=wq.ap()[:, sl, :].rearrange("h d e -> d h e")
                )
                nc.sync.dma_start(
                    out=wk_sb[:, dmc], in_=wk.ap()[:, sl, :].rearrange("h d e -> d h e")
                )
                nc.sync.dma_start(
                    out=wv_sb[:, dmc], in_=wv.ap()[:, sl, :].rearrange("h d e -> d h e")
                )
            wo_sb = consts.tile([128, H * DH // 128, D], F16, tag="wo")
            nc.sync.dma_start(
                out=wo_sb, in_=wo.ap().rearrange("(c p) d -> p c d", p=128)
            )

            bq_sb = consts.tile([128, 2], F32, tag="bq")
            bk_sb = consts.tile([128, 2], F32, tag="bk")
            for hp in range(2):
                nc.gpsimd.dma_start(
                    out=bq_sb[:, hp : hp + 1],
                    in_=bass.AP(tensor=bq.ap().tensor, offset=128 * hp, ap=[[1, 128], [1, 1]]),
                )
                nc.gpsimd.dma_start(
                    out=bk_sb[:, hp : hp + 1],
                    in_=bass.AP(tensor=bk.ap().tensor, offset=128 * hp, ap=[[1, 128], [1, 1]]),
                )
            bv_sb = consts.tile([128, HPC, DH], F32, tag="bv")
            nc.gpsimd.dma_start(
                out=bv_sb,
                in_=bass.AP(tensor=bv.ap().tensor, offset=0, ap=[[0, 128], [64, HPC], [1, DH]]),
            )
            bo_sb = consts.tile([128, D], F32, tag="bo")
            nc.gpsimd.dma_start(
                out=bo_sb,
                in_=bass.AP(tensor=bo.ap().tensor, offset=0, ap=[[0, 128], [1, D]]),
            )
            ones32 = consts.tile([1, DH], F32, tag="ones32")
            nc.sync.dma_start(out=ones32, in_=ones_dram.ap())
            ones_r = consts.tile([1, DH], F32R, tag="ones")
            nc.vector.tensor_copy(ones_r, ones32)

            # ---- projections -------------------------------------------------
            qT = persist.tile([128, 2, S], F16, tag="qT")  # [2 heads stacked, hp, pos]
            kT = persist.tile([128, 2, S], F16, tag="kT")
            v_aug = persist.tile([128, NKB, HPC, DH + 1], F16, tag="vaug")
            nc.vector.memset(v_aug[:, :, :, DH : DH + 1], 1.0)
            zT_sb = persist.tile([128, 2, S], F16, tag="zT")

            def project(x_dram, w_sb, kind, b_sb):
                for pc in range(NQC):
                    xt = work.tile([128, NDMC, QC], F16, tag="xt")
                    for dmc in range(NDMC):
                        nc.sync.dma_start(
                            out=xt[:, dmc],
                            in_=x_dram.ap()[
                                QC * pc : QC * (pc + 1), 128 * dmc : 128 * (dmc + 1)
                            ],
                            transpose=True,
                        )
                    if kind in ("q", "k"):
                        dst = qT if kind == "q" else kT
                        for hp in range(2):
                            pj = ps2.tile([128, QC], F32, tag="pqk")
                            lhs = w_sb[:, :, 2 * hp : 2 * hp + 2, :]
                            for dmc in range(NDMC):
                                nc.tensor.matmul(
                                    pj,
                                    lhs[:, dmc].rearrange("p h e -> p (h e)"),
                                    xt[:, dmc],
                                    start=(dmc == 0),
                                    stop=(dmc == NDMC - 1),
                                )
                            nc.vector.tensor_scalar_add(
                                dst[:, hp, QC * pc : QC * (pc + 1)],
                                pj,
                                b_sb[:, hp : hp + 1],
                            )
                    else:
                        for pb4 in range(4):
                            pv = ps2.tile([128, HPC * DH], F32, tag="pv")
                            for dmc in range(NDMC):
                                nc.tensor.matmul(
                                    pv,
                                    xt[:, dmc, 128 * pb4 : 128 * (pb4 + 1)],
                                    w_sb[:, dmc].rearrange("p h e -> p (h e)"),
                                    start=(dmc == 0),
                                    stop=(dmc == NDMC - 1),
                                )
                            kb = 4 * pc + pb4
                            nc.vector.tensor_add(
                                v_aug[:, kb, :, 0:DH],
                                pv.rearrange("p (h e) -> p h e", h=HPC),
                                bv_sb,
                            )

            project(xk, wk_sb, "k", bk_sb)
            project(xv, wv_sb, "v", None)
            project(xq, wq_sb, "q", bq_sb)

            # ---- attention ---------------------------------------------------
            for qc in range(NQC):
                for h in range(HPC):
                    hp, m0 = h // 2, 64 * (h % 2)
                    zps = ps.tile([DH + 1, QC], F32, tag="zps")
                    for kb in range(4 * qc + 4):
                        m = kb - 4 * qc
                        off = 0 if m < 0 else 128 * m
                        w = QC - off
                        st = ps2.tile([128, QC], F32, tag="st")
                        nc.tensor.matmul(
                            st[:, 0:w],
                            kT[m0 : m0 + 64, hp, 128 * kb : 128 * (kb + 1)],
                            qT[m0 : m0 + 64, hp, QC * qc + off : QC * (qc + 1)],
                            start=True,
                            stop=True,
                        )
                        pt = ptp.tile([128, QC], F16, tag="pt")
                        nc.scalar.activation(
                            pt[:, 0:w],
                            st[:, 0:w],
                            mybir.ActivationFunctionType.Exp,
                            scale=0.125,
                        )
                        if m >= 0:
                            nc.vector.tensor_mul(pt[:, 0:128], pt[:, 0:128], tri)
                        nc.tensor.matmul(
                            zps[:, off:QC],
                            v_aug[:, kb, h],
                            pt[:, 0:w],
                            start=(kb == 0),
                            stop=(kb == 4 * qc + 3),
                        )
                    rec32 = work.tile([1, QC], F32, tag="rec32")
                    nc.vector.reciprocal(rec32, zps[DH : DH + 1, :])
                    rec = work.tile([1, QC], F32R, tag="rec")
                    with nc.allow_low_precision(reason="f32r holds full fp32 bits"):
                        nc.vector.tensor_copy(rec, rec32)
                    bc = ps.tile([DH, QC], F32, tag="bc")
                    nc.tensor.matmul(bc, ones_r, rec, start=True, stop=True)
                    bcs = work.tile([DH, QC], F32, tag="bcs")
                    nc.scalar.copy(bcs, bc)
                    nc.vector.tensor_mul(
                        zT_sb[m0 : m0 + 64, hp, QC * qc : QC * (qc + 1)],
                        zps[0:DH, :],
                        bcs,
                    )

            # ---- allgather z^T + full output projection ---------------------
            z_dram = dram.tile([2, 128, S], F16, tag="zd")
            zfull_dram = dram.tile([4, 2, 128, S], F16, tag="zfd")
            for c in range(2):
                nc.sync.dma_start(out=z_dram[c], in_=zT_sb[:, c, :])
            nc.gpsimd.collective_compute(
                "AllGather",
                mybir.AluOpType.bypass,
                replica_groups=[[0, 1, 2, 3], [4, 5, 6, 7]],
                ins=[z_dram.opt()],
                outs=[zfull_dram.opt()],
            )

            for qb in range(S // 128):
                po0 = ps2.tile([128, 512], F32, tag="st")
                po1 = ps2.tile([128, 512], F32, tag="st")
                pos = (po0, po1)
                ob = work.tile([128, D], F32, tag="ob")
                for c in range(8):
                    zf = zfp.tile([128, 128], F16, tag="zf")
                    nc.sync.dma_start(
                        out=zf,
                        in_=zfull_dram[c // 2, c % 2, :, 128 * qb : 128 * (qb + 1)],
                    )
                    for dh2 in range(2):
                        nc.tensor.matmul(
                            pos[dh2],
                            zf,
                            wo_sb[:, c, 512 * dh2 : 512 * (dh2 + 1)],
                            start=(c == 0),
                            stop=(c == 7),
                        )
                for dh2 in range(2):
                    nc.vector.tensor_add(
                        ob[:, 512 * dh2 : 512 * (dh2 + 1)],
                        pos[dh2],
                        bo_sb[:, 512 * dh2 : 512 * (dh2 + 1)],
                    )
                nc.sync.dma_start(out=out.ap()[128 * qb : 128 * (qb + 1), :], in_=ob)

    nc.finalize()
    return nc


_CACHE = {}


def kernel(**inputs):
    _install_ntff_hook()
    nc = _CACHE.get("nc")
    if nc is None:
        nc = build()
        _CACHE["nc"] = nc

    f16 = np.float16
    xs = {k: np.asarray(inputs[k], np.float32) for k in ("query_input", "key_input", "value_input")}
    W = {k: np.asarray(inputs[k], np.float32) for k in ("W_Q", "W_K", "W_V", "W_O")}
    b = {k: np.asarray(inputs[k], np.float32) for k in ("b_Q", "b_K", "b_V", "b_O")}
    x16 = {k: np.ascontiguousarray(v).astype(f16) for k, v in xs.items()}
    wo16 = np.ascontiguousarray(W["W_O"].reshape(H * DH, D)).astype(f16)

    in_maps = []
    for i in range(N_CORES):
        g, h0 = i // 4, 4 * (i % 4)
        in_maps.append(
            {
                "xq": x16["query_input"][g],
                "xk": x16["key_input"][g],
                "xv": x16["value_input"][g],
                "wq": np.ascontiguousarray(W["W_Q"][h0 : h0 + HPC]).astype(f16),
                "wk": np.ascontiguousarray(W["W_K"][h0 : h0 + HPC]).astype(f16),
                "wv": np.ascontiguousarray(W["W_V"][h0 : h0 + HPC]).astype(f16),
                "wo": wo16,
                "bq": np.ascontiguousarray(b["b_Q"][h0 : h0 + HPC]),
                "bk": np.ascontiguousarray(b["b_K"][h0 : h0 + HPC]),
                "bv": np.ascontiguousarray(b["b_V"][h0 : h0 + HPC]),
                "bo": np.ascontiguousarray(b["b_O"]),
            }
        )

    res = run_bass_kernel_spmd(nc, in_maps, core_ids=list(range(N_CORES)))
    if os.environ.get("KERNEL_PRINT_EXEC"):
        print(f"HW exec time: {res.exec_time_ns} ns")
    return np.stack([res.results[0]["out"], res.results[4]["out"]], axis=0).astype(np.float32)


# revision 8
# speedup vs baseline: 1.1035x; 1.1035x over previous
"""Trainium2 Bass kernel for nn_AbstractAttention (B=2, S=2048, D=1024, H=16, dh=64).

Sharding: 8 cores = 2 batch groups x 4 cores. Core i handles batch i//4 and
heads 4*(i%4)..+4 for QKV projection + causal attention; z^T is AllGathered
(fp16, 1 MB/core) within each 4-core group and every core then runs the full
output projection; the host reads one core's output per batch.

Per core:
  - Q^T,K^T ([e,pos]) and V ([pos,e]) projections from DMA-transposed fp16
    inputs (host pre-casts to fp16; accumulation in fp32 PSUM).
  - per head: S^T[k,q] score tiles (only causal blocks), exp on ACT with the
    1/sqrt(dh) scale folded in, multiplicative triangular mask on diagonal
    blocks, unnormalized PV with a ones-column in V so z^T[64,:] is the
    softmax denominator; normalize via reciprocal + PE ones-broadcast.
  - AllGather z^T, out[2048,1024] = z_full^T.T @ W_O + b_O.
"""
import os, sys, types

sys.path.insert(0, "/opt/trn_rl_repo")
import numpy as np

import concourse.bass as bass
import concourse.bacc as bacc
import concourse.tile as tile
from concourse import mybir
from concourse.bass_utils import run_bass_kernel_spmd

B, S, D, H, DH = 2, 2048, 1024, 16, 64
N_CORES = 8
HPC = 4            # heads per core
QC = 512           # query chunk width for score tiles
NQC = S // QC      # 4
KB = 128           # key block
NKB = S // KB      # 16
NDMC = D // 128    # 8 d_model chunks
F16 = mybir.dt.float16
F32 = mybir.dt.float32
F32R = mybir.dt.float32r


def _install_ntff_hook():
    """Register the axon NTFF profiling hook missing from this image's antenv."""
    if "antenv.axon_hooks" in sys.modules:
        return
    try:
        from trn_agent_boot.trn_boot import _ntff_profile_via_ctypes

        hook = _ntff_profile_via_ctypes("/opt/axon/libaxon_pjrt.so")
        if hook is None:
            return
        import antenv  # noqa: F401

        mod = types.ModuleType("antenv.axon_hooks")
        mod.get_axon_ntff_profile_hook = lambda: hook
        sys.modules["antenv.axon_hooks"] = mod
    except Exception:
        pass


def build():
    nc = bacc.Bacc("TRN2", target_bir_lowering=False, debug=False, num_devices=N_CORES)
    xq = nc.dram_tensor("xq", [S, D], F16, kind="ExternalInput")
    xk = nc.dram_tensor("xk", [S, D], F16, kind="ExternalInput")
    xv = nc.dram_tensor("xv", [S, D], F16, kind="ExternalInput")
    wq = nc.dram_tensor("wq", [HPC, D, DH], F16, kind="ExternalInput")
    wk = nc.dram_tensor("wk", [HPC, D, DH], F16, kind="ExternalInput")
    wv = nc.dram_tensor("wv", [HPC, D, DH], F16, kind="ExternalInput")
    wo = nc.dram_tensor("wo", [H * DH, D], F16, kind="ExternalInput")
    bq = nc.dram_tensor("bq", [HPC, DH], F32, kind="ExternalInput")
    bk = nc.dram_tensor("bk", [HPC, DH], F32, kind="ExternalInput")
    bv = nc.dram_tensor("bv", [HPC, DH], F32, kind="ExternalInput")
    bo = nc.dram_tensor("bo", [D], F32, kind="ExternalInput")
    out = nc.dram_tensor("out", [S, D], F32, kind="ExternalOutput")

    tri_dram = nc.inline_tensor(np.triu(np.ones((128, 128), np.float16)), name="tri_c")
    ones_dram = nc.inline_tensor(np.ones((1, DH), np.float32), name="ones_c")

    with tile.TileContext(nc) as tc:
        with (
            tc.tile_pool(name="consts", bufs=1) as consts,
            tc.tile_pool(name="persist", bufs=1) as persist,
            tc.tile_pool(name="work", bufs=2) as work,
            tc.tile_pool(name="pt", bufs=3) as ptp,
            tc.tile_pool(name="zf", bufs=3) as zfp,
            tc.tile_pool(name="ps", bufs=1, space="PSUM") as ps,
            tc.tile_pool(name="ps2", bufs=2, space="PSUM") as ps2,
            tc.tile_pool(name="dram", bufs=1, space="DRAM") as dram,
        ):
            # ---- constants / weights / biases -------------------------------
            tri = consts.tile([128, 128], F16, tag="tri")
            nc.sync.dma_start(out=tri, in_=tri_dram.ap())

            wq_sb = consts.tile([128, NDMC, HPC, DH], F16, tag="wq")
            wk_sb = consts.tile([128, NDMC, HPC, DH], F16, tag="wk")
            wv_sb = consts.tile([128, NDMC, HPC, DH], F16, tag="wv")
            for dmc in range(NDMC):
                sl = slice(128 * dmc, 128 * (dmc + 1))
                nc.sync.dma_start(
                    out=wq_sb[:, dmc], in_=wq.ap()[:, sl, :].rearrange("h d e -> d h e")
                )
                nc.sync.dma_start(
                    out=wk_sb[:, dmc], in_=wk.ap()[:, sl, :].rearrange("h d e -> d h e")
                )
                nc.sync.dma_start(
                    out=wv_sb[:, dmc], in_=wv.ap()[:, sl, :].rearrange("h d e -> d h e")
                )
            wo_sb = consts.tile([128, H * DH // 128, D], F16, tag="wo")
            nc.sync.dma_start(
                out=wo_sb, in_=wo.ap().rearrange("(c p) d -> p c d", p=128)
            )

            bq_sb = consts.tile([128, 2], F32, tag="bq")
            bk_sb = consts.tile([128, 2], F32, tag="bk")
            for hp in range(2):
                nc.gpsimd.dma_start(
                    out=bq_sb[:, hp : hp + 1],
                    in_=bass.AP(tensor=bq.ap().tensor, offset=128 * hp, ap=[[1, 128], [1, 1]]),
                )
                nc.gpsimd.dma_start(
                    out=bk_sb[:, hp : hp + 1],
                    in_=bass.AP(tensor=bk.ap().tensor, offset=128 * hp, ap=[[1, 128], [1, 1]]),
                )
            bv_sb = consts.tile([128, HPC, DH], F32, tag="bv")
            nc.gpsimd.dma_start(
                out=bv_sb,
                in_=bass.AP(tensor=bv.ap().tensor, offset=0, ap=[[0, 128], [64, HPC], [1, DH]]),
            )
            bo_sb = consts.tile([128, D], F32, tag="bo")
            nc.gpsimd.dma_start(
                out=bo_sb,
                in_=bass.AP(tensor=bo.ap().tensor, offset=0, ap=[[0, 128], [1, D]]),
            )
            ones32 = consts.tile([1, DH], F32, tag="ones32")
            nc.sync.dma_start(out=ones32, in_=ones_dram.ap())
            ones_r = consts.tile([1, DH], F32R, tag="ones")
            nc.vector.tensor_copy(ones_r, ones32)

            # ---- projections -------------------------------------------------
            qT = persist.tile([128, 2, S], F16, tag="qT")  # [2 heads stacked, hp, pos]
            kT = persist.tile([128, 2, S], F16, tag="kT")
            v_aug = persist.tile([128, NKB, HPC, DH + 1], F16, tag="vaug")
            nc.vector.memset(v_aug[:, :, :, DH : DH + 1], 1.0)
            zT_sb = persist.tile([128, 2, S], F16, tag="zT")

            def project(x_dram, w_sb, kind, b_sb):
                for pc in range(NQC):
                    xt = work.tile([128, NDMC, QC], F16, tag="xt")
                    for dmc in range(NDMC):
                        nc.sync.dma_start(
                            out=xt[:, dmc],
                            in_=x_dram.ap()[
                                QC * pc : QC * (pc + 1), 128 * dmc : 128 * (dmc + 1)
                            ],
                            transpose=True,
                        )
                    if kind in ("q", "k"):
                        dst = qT if kind == "q" else kT
                        for hp in range(2):
                            pj = ps2.tile([128, QC], F32, tag="pqk")
                            lhs = w_sb[:, :, 2 * hp : 2 * hp + 2, :]
                            for dmc in range(NDMC):
                                nc.tensor.matmul(
                                    pj,
                                    lhs[:, dmc].rearrange("p h e -> p (h e)"),
                                    xt[:, dmc],
                                    start=(dmc == 0),
                                    stop=(dmc == NDMC - 1),
                                )
                            nc.vector.tensor_scalar_add(
                                dst[:, hp, QC * pc : QC * (pc + 1)],
                                pj,
                                b_sb[:, hp : hp + 1],
                            )
                    else:
                        for pb4 in range(4):
                            pv = ps2.tile([128, HPC * DH], F32, tag="pv")
                            for dmc in range(NDMC):
                                nc.tensor.matmul(
                                    pv,
                                    xt[:, dmc, 128 * pb4 : 128 * (pb4 + 1)],
                                    w_sb[:, dmc].rearrange("p h e -> p (h e)"),
                                    start=(dmc == 0),
                                    stop=(dmc == NDMC - 1),
                                )
                            kb = 4 * pc + pb4
                            nc.vector.tensor_add(
                                v_aug[:, kb, :, 0:DH],
                                pv.rearrange("p (h e) -> p h e", h=HPC),
                                bv_sb,
                            )

            project(xk, wk_sb, "k", bk_sb)
            project(xv, wv_sb, "v", None)
            project(xq, wq_sb, "q", bq_sb)

            # ---- attention ---------------------------------------------------
            for qc in range(NQC):
                for h in range(HPC):
                    hp, m0 = h // 2, 64 * (h % 2)
                    zps = ps.tile([DH + 1, QC], F32, tag="zps")
                    for kb in range(4 * qc + 4):
                        m = kb - 4 * qc
                        off = 0 if m < 0 else 128 * m
                        w = QC - off
                        st = ps2.tile([128, QC], F32, tag="st")
                        nc.tensor.matmul(
                            st[:, 0:w],
                            kT[m0 : m0 + 64, hp, 128 * kb : 128 * (kb + 1)],
                            qT[m0 : m0 + 64, hp, QC * qc + off : QC * (qc + 1)],
                            start=True,
                            stop=True,
                        )
                        pt = ptp.tile([128, QC], F16, tag="pt")
                        nc.scalar.activation(
                            pt[:, 0:w],
                            st[:, 0:w],
                            mybir.ActivationFunctionType.Exp,
                            scale=0.125,
                        )
                        if m >= 0:
                            nc.vector.tensor_mul(pt[:, 0:128], pt[:, 0:128], tri)
                        nc.tensor.matmul(
                            zps[:, off:QC],
                            v_aug[:, kb, h],
                            pt[:, 0:w],
                            start=(kb == 0),
                            stop=(kb == 4 * qc + 3),
                        )
                    rec32 = work.tile([1, QC], F32, tag="rec32")
                    nc.vector.reciprocal(rec32, zps[DH : DH + 1, :])
                    rec = work.tile([1, QC], F32R, tag="rec")
                    with nc.allow_low_precision(reason="f32r holds full fp32 bits"):
                        nc.vector.tensor_copy(rec, rec32)
                    bc = ps.tile([DH, QC], F32, tag="bc")
                    nc.tensor.matmul(bc, ones_r, rec, start=True, stop=True)
                    bcs = work.tile([DH, QC], F32, tag="bcs")
                    nc.scalar.copy(bcs, bc)
                    nc.vector.tensor_mul(
                        zT_sb[m0 : m0 + 64, hp, QC * qc : QC * (qc + 1)],
                        zps[0:DH, :],
                        bcs,
                    )

            # ---- allgather z^T + full output projection ---------------------
            z_dram = dram.tile([2, 128, S], F16, tag="zd")
            zfull_dram = dram.tile([4, 2, 128, S], F16, tag="zfd")
            for c in range(2):
                nc.sync.dma_start(out=z_dram[c], in_=zT_sb[:, c, :])
            nc.gpsimd.collective_compute(
                "AllGather",
                mybir.AluOpType.bypass,
                replica_groups=[[0, 1, 2, 3], [4, 5, 6, 7]],
                ins=[z_dram.opt()],
                outs=[zfull_dram.opt()],
            )

            zfull_sb = persist.tile([128, 8, S], F16, tag="zfull")
            for c in range(8):
                nc.sync.dma_start(out=zfull_sb[:, c], in_=zfull_dram[c // 2, c % 2])
            for qb in range(S // 128):
                po0 = ps2.tile([128, 512], F32, tag="st")
                po1 = ps2.tile([128, 512], F32, tag="st")
                pos = (po0, po1)
                ob = work.tile([128, D], F32, tag="ob")
                for c in range(8):
                    for dh2 in range(2):
                        nc.tensor.matmul(
                            pos[dh2],
                            zfull_sb[:, c, 128 * qb : 128 * (qb + 1)],
                            wo_sb[:, c, 512 * dh2 : 512 * (dh2 + 1)],
                            start=(c == 0),
                            stop=(c == 7),
                        )
                for dh2 in range(2):
                    nc.vector.tensor_add(
                        ob[:, 512 * dh2 : 512 * (dh2 + 1)],
                        pos[dh2],
                        bo_sb[:, 512 * dh2 : 512 * (dh2 + 1)],
                    )
                nc.sync.dma_start(out=out.ap()[128 * qb : 128 * (qb + 1), :], in_=ob)

    nc.finalize()
    return nc


_CACHE = {}


def kernel(**inputs):
    _install_ntff_hook()
    nc = _CACHE.get("nc")
    if nc is None:
        nc = build()
        _CACHE["nc"] = nc

    f16 = np.float16
    xs = {k: np.asarray(inputs[k], np.float32) for k in ("query_input", "key_input", "value_input")}
    W = {k: np.asarray(inputs[k], np.float32) for k in ("W_Q", "W_K", "W_V", "W_O")}
    b = {k: np.asarray(inputs[k], np.float32) for k in ("b_Q", "b_K", "b_V", "b_O")}
    x16 = {k: np.ascontiguousarray(v).astype(f16) for k, v in xs.items()}
    wo16 = np.ascontiguousarray(W["W_O"].reshape(H * DH, D)).astype(f16)

    in_maps = []
    for i in range(N_CORES):
        g, h0 = i // 4, 4 * (i % 4)
        in_maps.append(
            {
                "xq": x16["query_input"][g],
                "xk": x16["key_input"][g],
                "xv": x16["value_input"][g],
                "wq": np.ascontiguousarray(W["W_Q"][h0 : h0 + HPC]).astype(f16),
                "wk": np.ascontiguousarray(W["W_K"][h0 : h0 + HPC]).astype(f16),
                "wv": np.ascontiguousarray(W["W_V"][h0 : h0 + HPC]).astype(f16),
                "wo": wo16,
                "bq": np.ascontiguousarray(b["b_Q"][h0 : h0 + HPC]),
                "bk": np.ascontiguousarray(b["b_K"][h0 : h0 + HPC]),
                "bv": np.ascontiguousarray(b["b_V"][h0 : h0 + HPC]),
                "bo": np.ascontiguousarray(b["b_O"]),
            }
        )

    res = run_bass_kernel_spmd(nc, in_maps, core_ids=list(range(N_CORES)))
    if os.environ.get("KERNEL_PRINT_EXEC"):
        print(f"HW exec time: {res.exec_time_ns} ns")
    return np.stack([res.results[0]["out"], res.results[4]["out"]], axis=0).astype(np.float32)


# revision 10
# speedup vs baseline: 1.2747x; 1.1552x over previous
"""Trainium2 Bass kernel for nn_AbstractAttention (B=2, S=2048, D=1024, H=16, dh=64).

Sharding: 8 cores = 2 batch groups x 4 cores. Core i handles batch i//4 and
heads 4*(i%4)..+4 for QKV projection + causal attention; z^T is AllGathered
(fp16, 1 MB/core) within each 4-core group and every core then runs the full
output projection; the host reads one core's output per batch.

Per core:
  - Q^T,K^T ([e,pos]) and V ([pos,e]) projections from DMA-transposed fp16
    inputs (host pre-casts to fp16; accumulation in fp32 PSUM).
  - per head: S^T[k,q] score tiles (only causal blocks), exp on ACT with the
    1/sqrt(dh) scale folded in, multiplicative triangular mask on diagonal
    blocks, unnormalized PV with a ones-column in V so z^T[64,:] is the
    softmax denominator; normalize via reciprocal + PE ones-broadcast.
  - AllGather z^T, out[2048,1024] = z_full^T.T @ W_O + b_O.
"""
import os, sys, types

sys.path.insert(0, "/opt/trn_rl_repo")
import numpy as np

import concourse.bass as bass
import concourse.bacc as bacc
import concourse.tile as tile
from concourse import mybir
from concourse.bass_utils import run_bass_kernel_spmd

B, S, D, H, DH = 2, 2048, 1024, 16, 64
N_CORES = 8
HPC = 4            # heads per core
QC = 512           # query chunk width for score tiles
NQC = S // QC      # 4
KB = 128           # key block
NKB = S // KB      # 16
NDMC = D // 128    # 8 d_model chunks
F16 = mybir.dt.float16
F32 = mybir.dt.float32
F32R = mybir.dt.float32r


def _install_ntff_hook():
    """Register the axon NTFF profiling hook missing from this image's antenv."""
    if "antenv.axon_hooks" in sys.modules:
        return
    try:
        from trn_agent_boot.trn_boot import _ntff_profile_via_ctypes

        hook = _ntff_profile_via_ctypes("/opt/axon/libaxon_pjrt.so")
        if hook is None:
            return
        import antenv  # noqa: F401

        mod = types.ModuleType("antenv.axon_hooks")
        mod.get_axon_ntff_profile_hook = lambda: hook
        sys.modules["antenv.axon_hooks"] = mod
    except Exception:
        pass


def build():
    nc = bacc.Bacc("TRN2", target_bir_lowering=False, debug=False, num_devices=N_CORES)
    xq = nc.dram_tensor("xq", [S, D], F16, kind="ExternalInput")
    xk = nc.dram_tensor("xk", [S, D], F16, kind="ExternalInput")
    xv = nc.dram_tensor("xv", [S, D], F16, kind="ExternalInput")
    wq = nc.dram_tensor("wq", [HPC, D, DH], F16, kind="ExternalInput")
    wk = nc.dram_tensor("wk", [HPC, D, DH], F16, kind="ExternalInput")
    wv = nc.dram_tensor("wv", [HPC, D, DH], F16, kind="ExternalInput")
    wo = nc.dram_tensor("wo", [H * DH, D], F16, kind="ExternalInput")
    bq = nc.dram_tensor("bq", [HPC, DH], F32, kind="ExternalInput")
    bk = nc.dram_tensor("bk", [HPC, DH], F32, kind="ExternalInput")
    bv = nc.dram_tensor("bv", [HPC, DH], F32, kind="ExternalInput")
    bo = nc.dram_tensor("bo", [D], F32, kind="ExternalInput")
    out = nc.dram_tensor("out", [S, D], F32, kind="ExternalOutput")

    tri_dram = nc.inline_tensor(np.triu(np.ones((128, 128), np.float16)), name="tri_c")
    ones_dram = nc.inline_tensor(np.ones((1, DH), np.float32), name="ones_c")

    with tile.TileContext(nc) as tc:
        with (
            tc.tile_pool(name="consts", bufs=1) as consts,
            tc.tile_pool(name="persist", bufs=1) as persist,
            tc.tile_pool(name="work", bufs=2) as work,
            tc.tile_pool(name="pt", bufs=3) as ptp,
            tc.tile_pool(name="zf", bufs=3) as zfp,
            tc.tile_pool(name="ps", bufs=1, space="PSUM") as ps,
            tc.tile_pool(name="ps2", bufs=2, space="PSUM") as ps2,
            tc.tile_pool(name="ps3", bufs=3, space="PSUM") as ps3,
            tc.tile_pool(name="dram", bufs=1, space="DRAM") as dram,
        ):
            # ---- constants / weights / biases -------------------------------
            tri = consts.tile([128, 128], F16, tag="tri")
            nc.sync.dma_start(out=tri, in_=tri_dram.ap())

            wq_sb = consts.tile([128, NDMC, HPC, DH], F16, tag="wq")
            wk_sb = consts.tile([128, NDMC, HPC, DH], F16, tag="wk")
            wv_sb = consts.tile([128, NDMC, HPC, DH], F16, tag="wv")
            for dmc in range(NDMC):
                sl = slice(128 * dmc, 128 * (dmc + 1))
                nc.sync.dma_start(
                    out=wq_sb[:, dmc], in_=wq.ap()[:, sl, :].rearrange("h d e -> d h e")
                )
                nc.sync.dma_start(
                    out=wk_sb[:, dmc], in_=wk.ap()[:, sl, :].rearrange("h d e -> d h e")
                )
                nc.sync.dma_start(
                    out=wv_sb[:, dmc], in_=wv.ap()[:, sl, :].rearrange("h d e -> d h e")
                )
            wo_sb = consts.tile([128, H * DH // 128, D], F16, tag="wo")
            nc.sync.dma_start(
                out=wo_sb, in_=wo.ap().rearrange("(c p) d -> p c d", p=128)
            )

            bq_sb = consts.tile([128, 2], F32, tag="bq")
            bk_sb = consts.tile([128, 2], F32, tag="bk")
            for hp in range(2):
                nc.gpsimd.dma_start(
                    out=bq_sb[:, hp : hp + 1],
                    in_=bass.AP(tensor=bq.ap().tensor, offset=128 * hp, ap=[[1, 128], [1, 1]]),
                )
                nc.gpsimd.dma_start(
                    out=bk_sb[:, hp : hp + 1],
                    in_=bass.AP(tensor=bk.ap().tensor, offset=128 * hp, ap=[[1, 128], [1, 1]]),
                )
            bv_sb = consts.tile([128, HPC, DH], F32, tag="bv")
            nc.gpsimd.dma_start(
                out=bv_sb,
                in_=bass.AP(tensor=bv.ap().tensor, offset=0, ap=[[0, 128], [64, HPC], [1, DH]]),
            )
            bo_sb = consts.tile([128, D], F32, tag="bo")
            nc.gpsimd.dma_start(
                out=bo_sb,
                in_=bass.AP(tensor=bo.ap().tensor, offset=0, ap=[[0, 128], [1, D]]),
            )
            ones32 = consts.tile([1, DH], F32, tag="ones32")
            nc.sync.dma_start(out=ones32, in_=ones_dram.ap())
            ones_r = consts.tile([1, DH], F32R, tag="ones")
            nc.vector.tensor_copy(ones_r, ones32)

            # ---- projections -------------------------------------------------
            qT = persist.tile([128, 2, S], F16, tag="qT")  # [2 heads stacked, hp, pos]
            kT = persist.tile([128, 2, S], F16, tag="kT")
            v_aug = persist.tile([128, NKB, HPC, DH + 1], F16, tag="vaug")
            nc.vector.memset(v_aug[:, :, :, DH : DH + 1], 1.0)
            zT_sb = persist.tile([128, 2, S], F16, tag="zT")

            def project(x_dram, w_sb, kind, b_sb):
                for pc in range(NQC):
                    xt = work.tile([128, NDMC, QC], F16, tag="xt")
                    for dmc in range(NDMC):
                        nc.sync.dma_start(
                            out=xt[:, dmc],
                            in_=x_dram.ap()[
                                QC * pc : QC * (pc + 1), 128 * dmc : 128 * (dmc + 1)
                            ],
                            transpose=True,
                        )
                    if kind in ("q", "k"):
                        dst = qT if kind == "q" else kT
                        for hp in range(2):
                            pj = ps2.tile([128, QC], F32, tag="pj")
                            lhs = w_sb[:, :, 2 * hp : 2 * hp + 2, :]
                            for dmc in range(NDMC):
                                nc.tensor.matmul(
                                    pj,
                                    lhs[:, dmc].rearrange("p h e -> p (h e)"),
                                    xt[:, dmc],
                                    start=(dmc == 0),
                                    stop=(dmc == NDMC - 1),
                                )
                            nc.vector.tensor_scalar_add(
                                dst[:, hp, QC * pc : QC * (pc + 1)],
                                pj,
                                b_sb[:, hp : hp + 1],
                            )
                    else:
                        for pb4 in range(4):
                            pv_full = ps2.tile([128, QC], F32, tag="pj")
                            pv = pv_full[:, 0 : HPC * DH]
                            for dmc in range(NDMC):
                                nc.tensor.matmul(
                                    pv,
                                    xt[:, dmc, 128 * pb4 : 128 * (pb4 + 1)],
                                    w_sb[:, dmc].rearrange("p h e -> p (h e)"),
                                    start=(dmc == 0),
                                    stop=(dmc == NDMC - 1),
                                )
                            kb = 4 * pc + pb4
                            nc.vector.tensor_add(
                                v_aug[:, kb, :, 0:DH],
                                pv.rearrange("p (h e) -> p h e", h=HPC),
                                bv_sb,
                            )

            project(xk, wk_sb, "k", bk_sb)
            project(xv, wv_sb, "v", None)
            project(xq, wq_sb, "q", bq_sb)

            # ---- attention + incremental allgather --------------------------
            z_dram = dram.tile([2, 128, S], F16, tag="zd")
            zfA = dram.tile([4, 128, S], F16, tag="zfa")
            zfB = dram.tile([4, 128, S], F16, tag="zfb")
            for h in range(HPC):
                for qc in range(NQC):
                    hp, m0 = h // 2, 64 * (h % 2)
                    zps = ps2.tile([DH + 1, QC], F32, tag="zps")
                    for kb in range(4 * qc + 4):
                        m = kb - 4 * qc
                        off = 0 if m < 0 else 128 * m
                        w = QC - off
                        st = ps3.tile([128, QC], F32, tag="st")
                        nc.tensor.matmul(
                            st[:, 0:w],
                            kT[m0 : m0 + 64, hp, 128 * kb : 128 * (kb + 1)],
                            qT[m0 : m0 + 64, hp, QC * qc + off : QC * (qc + 1)],
                            start=True,
                            stop=True,
                        )
                        pt = ptp.tile([128, QC], F16, tag="pt")
                        nc.scalar.activation(
                            pt[:, 0:w],
                            st[:, 0:w],
                            mybir.ActivationFunctionType.Exp,
                            scale=0.125,
                        )
                        if m >= 0:
                            nc.vector.tensor_mul(pt[:, 0:128], pt[:, 0:128], tri)
                        nc.tensor.matmul(
                            zps[:, off:QC],
                            v_aug[:, kb, h],
                            pt[:, 0:w],
                            start=(kb == 0),
                            stop=(kb == 4 * qc + 3),
                        )
                    rec32 = work.tile([1, QC], F32, tag="rec32")
                    nc.vector.reciprocal(rec32, zps[DH : DH + 1, :])
                    rec = work.tile([1, QC], F32R, tag="rec")
                    with nc.allow_low_precision(reason="f32r holds full fp32 bits"):
                        nc.vector.tensor_copy(rec, rec32)
                    bc = ps.tile([DH, QC], F32, tag="bc")
                    nc.tensor.matmul(bc, ones_r, rec, start=True, stop=True)
                    bcs = work.tile([DH, QC], F32, tag="bcs")
                    nc.scalar.copy(bcs, bc)
                    nc.vector.tensor_mul(
                        zT_sb[m0 : m0 + 64, hp, QC * qc : QC * (qc + 1)],
                        zps[0:DH, :],
                        bcs,
                    )

                if h % 2 == 1:
                    c = h // 2
                    nc.sync.dma_start(out=z_dram[c], in_=zT_sb[:, c, :])
                    nc.gpsimd.collective_compute(
                        "AllGather",
                        mybir.AluOpType.bypass,
                        replica_groups=[[0, 1, 2, 3], [4, 5, 6, 7]],
                        ins=[z_dram[c].opt()],
                        outs=[(zfA if c == 0 else zfB).opt()],
                    )

            zfull_sb = persist.tile([128, 8, S], F16, tag="zfull")
            for c in range(8):
                src_t = zfA if c % 2 == 0 else zfB
                nc.sync.dma_start(out=zfull_sb[:, c], in_=src_t[c // 2])
            for qb in range(S // 128):
                po0 = ps3.tile([128, 512], F32, tag="st")
                po1 = ps3.tile([128, 512], F32, tag="st")
                pos = (po0, po1)
                ob = work.tile([128, D], F32, tag="ob")
                for c in range(8):
                    for dh2 in range(2):
                        nc.tensor.matmul(
                            pos[dh2],
                            zfull_sb[:, c, 128 * qb : 128 * (qb + 1)],
                            wo_sb[:, c, 512 * dh2 : 512 * (dh2 + 1)],
                            start=(c == 0),
                            stop=(c == 7),
                        )
                for dh2 in range(2):
                    nc.vector.tensor_add(
                        ob[:, 512 * dh2 : 512 * (dh2 + 1)],
                        pos[dh2],
                        bo_sb[:, 512 * dh2 : 512 * (dh2 + 1)],
                    )
                nc.sync.dma_start(out=out.ap()[128 * qb : 128 * (qb + 1), :], in_=ob)

    nc.finalize()
    return nc


_CACHE = {}


def kernel(**inputs):
    _install_ntff_hook()
    nc = _CACHE.get("nc")
    if nc is None:
        nc = build()
        _CACHE["nc"] = nc

    f16 = np.float16
    xs = {k: np.asarray(inputs[k], np.float32) for k in ("query_input", "key_input", "value_input")}
    W = {k: np.asarray(inputs[k], np.float32) for k in ("W_Q", "W_K", "W_V", "W_O")}
    b = {k: np.asarray(inputs[k], np.float32) for k in ("b_Q", "b_K", "b_V", "b_O")}
    x16 = {k: np.ascontiguousarray(v).astype(f16) for k, v in xs.items()}
    wo16 = np.ascontiguousarray(W["W_O"].reshape(H * DH, D)).astype(f16)

    in_maps = []
    for i in range(N_CORES):
        g, h0 = i // 4, 4 * (i % 4)
        in_maps.append(
            {
                "xq": x16["query_input"][g],
                "xk": x16["key_input"][g],
                "xv": x16["value_input"][g],
                "wq": np.ascontiguousarray(W["W_Q"][h0 : h0 + HPC]).astype(f16),
                "wk": np.ascontiguousarray(W["W_K"][h0 : h0 + HPC]).astype(f16),
                "wv": np.ascontiguousarray(W["W_V"][h0 : h0 + HPC]).astype(f16),
                "wo": wo16,
                "bq": np.ascontiguousarray(b["b_Q"][h0 : h0 + HPC]),
                "bk": np.ascontiguousarray(b["b_K"][h0 : h0 + HPC]),
                "bv": np.ascontiguousarray(b["b_V"][h0 : h0 + HPC]),
                "bo": np.ascontiguousarray(b["b_O"]),
            }
        )

    res = run_bass_kernel_spmd(nc, in_maps, core_ids=list(range(N_CORES)))
    if os.environ.get("KERNEL_PRINT_EXEC"):
        print(f"HW exec time: {res.exec_time_ns} ns")
    return np.stack([res.results[0]["out"], res.results[4]["out"]], axis=0).astype(np.float32)
